# revision 2
# baseline (speedup 1.0000x reference)
"""Trainium2 Bass kernel for the dMaSIFConvBlock problem (bf16, v8).

Effective math (points/nuv/ranges are dead inputs in the reference):
    out = relu(features @ W1.T + b1) @ Wb.T + bb        (W1 = Wa@Wt fused)
a pointwise 16->16->16 MLP over 2M points.  All HBM traffic moves as
bf16 (the 2e-2 rel-err budget is ~40x above bf16 round-off): ~8 MB in
+ ~8 MB out per core at ~358 GB/s -> ~46 us/core memory floor.

Per-core pipeline (points split 8 ways, weights replicated):

  - The host marshals each core's shard into the channel-major bundle
    layout while it pre-rounds to bf16 (one fused strided numpy pass):
    HBM element p*cols + r holds channel p%16 of point 8r + p//16.
    Device loads are therefore plain fully-contiguous 1 MB DMAs that
    arrive matmul-ready -- no on-chip or DMA-xbar transpose at all.
    (The xbar DMA-transpose path was measured at ~240 GB/s alone and
    ~140 GB/s under concurrent store traffic due to its 256 B
    descriptors, and the Tile scheduler serializes it against all other
    DMA queues as a deadlock guard; plain loads run near fabric rate
    and overlap stores freely.)
  - 16x16 weights packed 8x along the diagonal of a 128x128 bf16
    stationary; an N=512 matmul applies one layer to 4096 points.
  - Chunks (4096 columns = 32768 points) are software-pipelined on the
    PE: layer-1 of chunk c+1 issues between layer-1 and layer-2 of
    chunk c, so every PSUM reuse has a full 8-matmul run of slack and
    the PE stays dense (HAM keeps the PE clock at full rate).
  - PSUM tiles are [128,1024] f32 (two banks, two matmuls each); one
    post-op drains two superblocks.  The 62 post-ops (bias+ReLU /
    bias) alternate between ScalarE and DVE -- the PSUM-evacuation
    floor (~37 us/engine; TRN2 engines read PSUM f32 at 1/lane/cycle).
  - Output stays channel-major, stored as contiguous 512 KB DMAs on
    the gpsimd/SWDGE ring; the host undoes the interleave during
    unsharding.

One environment quirk is handled at build time: this walrus build
rejects instructions with more than one semaphore wait, while the Tile
scheduler freely attaches several; _split_multi_waits moves every
extra wait onto a standalone NoOp.
"""

import ml_dtypes
import numpy as np

import concourse.bass as bass
import concourse.tile as tile
from concourse import mybir
from concourse.bass_utils import run_bass_kernel_spmd

N_TOTAL = 2_000_000
C = 16
N_CORES = 8
N_SHARD = N_TOTAL // N_CORES       # 250_000 points per core
CHUNK_DBS = [4] * 7 + [3]          # 1024-col double-superblocks per chunk
N_CHUNKS = len(CHUNK_DBS)
N_PAD = sum(CHUNK_DBS) * 8192      # 253_952 points per core (1.6% padding)
CHUNK_FREE = 4096                  # bf16 per partition, full chunk

F32 = mybir.dt.float32
BF16 = mybir.dt.bfloat16
BF16_NP = ml_dtypes.bfloat16


def _split_multi_waits(nc):
    """Walrus here allows at most one semaphore wait per instruction.
    Move every extra wait onto its own NoOp placed just before the
    instruction on the same engine (waiting earlier on the same engine
    is equivalent: the waits' producers are other engines/queues)."""
    for func in nc.m.functions:
        for bb in func.blocks:
            out = []
            changed = False
            for inst in bb.instructions:
                si = inst.sync_info
                if si is not None and len(si.on_wait) > 1:
                    waits = list(si.on_wait)
                    for j, w in enumerate(waits[:-1]):
                        out.append(
                            mybir.InstNoOp(
                                name=f"{inst.name}-xw{j}",
                                sync_info=mybir.SyncInfo(on_wait=[w], on_update=[]),
                                bass_nofuse=True,
                                engine=inst.engine,
                            )
                        )
                    si.on_wait = [waits[-1]]
                    inst.sync_info = si
                    changed = True
                out.append(inst)
            if changed:
                bb.instructions = out


def _build_program():
    nc = bass.Bass()
    x_d = nc.dram_tensor("x", [N_PAD * C], BF16, kind="ExternalInput")
    y_d = nc.dram_tensor("y", [N_PAD * C], BF16, kind="ExternalOutput")
    w1_d = nc.dram_tensor("bdw1", [128, 128], BF16, kind="ExternalInput")
    wb_d = nc.dram_tensor("bdwb", [128, 128], BF16, kind="ExternalInput")
    b1_d = nc.dram_tensor("b1p", [128, 1], F32, kind="ExternalInput")
    b2_d = nc.dram_tensor("b2p", [128, 1], F32, kind="ExternalInput")

    # Per-chunk [128, cols] views; both input and output are partition-
    # major contiguous in the channel-major bundle layout.
    x_v, y_v = [], []
    base = 0
    for ndb in CHUNK_DBS:
        cols = ndb * 1024
        n_el = 128 * cols
        x_v.append(x_d.ap()[base : base + n_el].rearrange("(p m) -> p m", p=128))
        y_v.append(y_d.ap()[base : base + n_el].rearrange("(p m) -> p m", p=128))
        base += n_el
    relu = mybir.ActivationFunctionType.Relu
    add_op = mybir.AluOpType.add
    max_op = mybir.AluOpType.max

    with tile.TileContext(nc) as tc:
        with (
            tc.tile_pool(name="consts", bufs=1) as consts,
            tc.tile_pool(name="xpool", bufs=8) as xpool,
            tc.tile_pool(name="ypool", bufs=4) as ypool,
            tc.tile_pool(name="work", bufs=8) as work,
            tc.tile_pool(name="psum", bufs=4, space="PSUM") as psum,
        ):
            bdw1 = consts.tile([128, 128], BF16)
            nc.scalar.dma_start(bdw1[:], w1_d.ap())
            b1p = consts.tile([128, 1], F32)
            nc.scalar.dma_start(b1p[:], b1_d.ap())
            bdwb = consts.tile([128, 128], BF16)
            nc.scalar.dma_start(bdwb[:], wb_d.ap())
            b2p = consts.tile([128, 1], F32)
            nc.scalar.dma_start(b2p[:], b2_d.ap())

            rrbox = [0]

            def post_l1(yb_sl, h):
                if rrbox[0] % 2 == 0:
                    nc.scalar.activation(yb_sl, h[:], relu, bias=b1p[:])
                else:
                    nc.vector.tensor_scalar(yb_sl, h[:], b1p[:], 0.0, add_op, max_op)
                rrbox[0] += 1

            def post_l2(ys_sl, h):
                if rrbox[0] % 2 == 0:
                    nc.scalar.add(ys_sl, h[:], b2p[:])
                else:
                    nc.vector.tensor_scalar_add(ys_sl, h[:], b2p[:])
                rrbox[0] += 1

            xs_t, ybs_t = {}, {}

            def load(c):
                xs = xpool.tile([128, CHUNK_FREE], BF16, tag="xs")
                cols = CHUNK_DBS[c] * 1024
                if c == 0:
                    hf = cols // 2
                    nc.sync.dma_start(xs[:, :hf], x_v[c][:, :hf])
                    nc.sync.dma_start(xs[:, hf:cols], x_v[c][:, hf:cols])
                else:
                    nc.sync.dma_start(xs[:, :cols], x_v[c])
                xs_t[c] = xs

            def l1_run(c):
                xs = xs_t.pop(c)
                ndb = CHUNK_DBS[c]
                hs = []
                for i in range(ndb):
                    c0 = 1024 * i
                    h = psum.tile([128, 1024], F32, tag="h")
                    nc.tensor.matmul(h[:, :512], bdw1[:], xs[:, c0 : c0 + 512])
                    nc.tensor.matmul(h[:, 512:], bdw1[:], xs[:, c0 + 512 : c0 + 1024])
                    hs.append(h)
                ybs = []
                for h in hs:
                    yb = work.tile([128, 1024], BF16, tag="yb")
                    post_l1(yb[:], h)
                    ybs.append(yb)
                ybs_t[c] = ybs

            def l2_run(c):
                ybs = ybs_t.pop(c)
                ndb = CHUNK_DBS[c]
                cols = ndb * 1024
                hf = (ndb // 2) * 1024
                ys = ypool.tile([128, CHUNK_FREE], BF16, tag="ys")
                hs = []
                for i in range(ndb):
                    h = psum.tile([128, 1024], F32, tag="h")
                    nc.tensor.matmul(h[:, :512], bdwb[:], ybs[i][:, :512])
                    nc.tensor.matmul(h[:, 512:], bdwb[:], ybs[i][:, 512:])
                    hs.append(h)
                for i, h in enumerate(hs):
                    c0 = 1024 * i
                    post_l2(ys[:, c0 : c0 + 1024], h)
                    if hf and c0 + 1024 == hf:
                        nc.gpsimd.dma_start(y_v[c][:, :hf], ys[:, :hf])
                nc.gpsimd.dma_start(y_v[c][:, hf:], ys[:, hf:cols])

            for c in range(N_CHUNKS):
                load(c)
            l1_run(0)
            for c in range(N_CHUNKS):
                if c + 1 < N_CHUNKS:
                    l1_run(c + 1)
                l2_run(c)

    _split_multi_waits(nc)
    return nc


_NC = None


def _get_program():
    global _NC
    if _NC is None:
        _NC = _build_program()
    return _NC


def _encode_in(feats_core):
    """Marshal one core's [N_PAD, 16] f32 shard into the channel-major
    bundle layout as bf16: flat element p*cols + r holds channel p%16
    of point 8r + p//16 (per chunk)."""
    out = np.empty(N_PAD * C, BF16_NP)
    base_pt = 0
    base_el = 0
    for ndb in CHUNK_DBS:
        cols = ndb * 1024
        pts = cols * 8
        n_el = 128 * cols
        chunk = feats_core[base_pt : base_pt + pts].reshape(cols, 8, 16)
        out[base_el : base_el + n_el] = (
            np.transpose(chunk, (1, 2, 0)).astype(BF16_NP).reshape(-1)
        )
        base_pt += pts
        base_el += n_el
    return out


def _prepare_in_maps(inputs):
    feats = np.ascontiguousarray(np.asarray(inputs["features"], dtype=np.float32))
    Wt = np.asarray(inputs["Wt"], dtype=np.float32)
    bt = np.asarray(inputs["bt"], dtype=np.float32)
    Wa = np.asarray(inputs["Wa"], dtype=np.float32)
    ba = np.asarray(inputs["ba"], dtype=np.float32)
    Wb = np.asarray(inputs["Wb"], dtype=np.float32)
    bb = np.asarray(inputs["bb"], dtype=np.float32)

    W1 = (Wa @ Wt).astype(np.float32)
    b1 = (Wa @ bt + ba).astype(np.float32)

    bdw1 = np.zeros((128, 128), np.float32)
    bdwb = np.zeros((128, 128), np.float32)
    for g in range(8):
        bdw1[16 * g : 16 * g + 16, 16 * g : 16 * g + 16] = W1.T
        bdwb[16 * g : 16 * g + 16, 16 * g : 16 * g + 16] = Wb.T
    bdw1 = bdw1.astype(BF16_NP)
    bdwb = bdwb.astype(BF16_NP)
    b1p = np.tile(b1, 8).astype(np.float32).reshape(128, 1)
    b2p = np.tile(bb, 8).astype(np.float32).reshape(128, 1)

    feats = feats.reshape(N_CORES, N_SHARD, C)
    pad = np.zeros((N_PAD - N_SHARD, C), np.float32)
    return [
        {
            "x": _encode_in(np.concatenate([feats[i], pad], axis=0)),
            "bdw1": bdw1,
            "bdwb": bdwb,
            "b1p": b1p,
            "b2p": b2p,
        }
        for i in range(N_CORES)
    ]


def _decode_out(y_flat):
    """Undo the device's channel-major bundle layout for one core."""
    parts = []
    base = 0
    for ndb in CHUNK_DBS:
        cols = ndb * 1024
        n_el = 128 * cols
        seg = y_flat[base : base + n_el].reshape(8, 16, cols)
        parts.append(np.transpose(seg, (2, 0, 1)).reshape(cols * 8, 16))
        base += n_el
    return np.concatenate(parts, axis=0)[:N_SHARD]


def _run(inputs, trace=False):
    nc = _get_program()
    in_maps = _prepare_in_maps(inputs)
    res = run_bass_kernel_spmd(nc, in_maps, core_ids=list(range(N_CORES)), trace=trace)
    parts = [_decode_out(res.results[i]["y"]) for i in range(N_CORES)]
    out = np.concatenate(parts, axis=0).astype(np.float32)
    return out, res


def kernel(**inputs) -> np.ndarray:
    out, _ = _run(inputs, trace=False)
    return out
